# Initial kernel scaffold
#
"""LLaDA2 MoE decoder layer as a single SPMD Bass/Tile kernel on 8 TRN2 cores.

Sharding:
  - Attention: core c handles batch b = c//4 and kv-group kv = c%4
    (1 kv head + its 4 q heads, full 1024-token sequence of batch b).
    A 4-core ReduceScatter sums the w_dense partial products and hands each
    core its 256-token slice (global tokens [256c, 256c+256)).
  - MoE: expert-parallel. Each core owns expert e = c. x2 (post-ln2
    activations, transposed) + routing weights are AllGathered so every core
    runs its expert over all 2048 tokens; partial expert outputs are summed
    and token-scattered with an 8-core ReduceScatter. Residual + shared
    expert are added per-owner after the RS.

Layouts: activations mostly live in "T layout" [feature(partitions), token
(free)] so matmul contraction runs over partitions with no on-device weight
transposes (all weights are pre-transposed on the host, which is free).
"""

import math
import os

import numpy as np

import concourse.bass as bass
import concourse.mybir as mybir
import concourse.tile as tile
from concourse import bacc
from concourse.masks import make_identity

# Problem shapes (hardcoded per contest rules).
B, S, H = 2, 1024, 2048
NH, NKV, HD = 16, 4, 128
E, TOPK, NG, TOPKG, M = 8, 2, 4, 2, 512
EPS = 1e-6
SCALE = HD ** -0.5
T = B * S                      # 2048 tokens
NCORES = 8
TOK = T // NCORES              # 256 tokens owned per core
QH = NH // NKV                 # 4 q heads per kv head
GSZ = E // NG                  # experts per group = 2
P = 128
F32 = mybir.dt.float32
F32R = mybir.dt.float32r
BIG = 1e30

HC = H // P                    # 16 h chunks
SC = S // P                    # 8 seq chunks per batch
MC = M // P                    # 4 m chunks
TC = TOK // P                  # 2 own-token chunks
NT = H // 512                  # 4 512-wide h' tiles
TBLK = NCORES                  # 8 token blocks of 256 after AG
AGROWS = H + E                 # 2056 rows in allgather buffer


def r32(ap):
    return ap.bitcast(F32R)


def _mm(nc, out, lhsT, rhs, start, stop):
    nc.tensor.matmul(out, r32(lhsT), r32(rhs), start=start, stop=stop)


def _dma_r(nc, out, in_):
    nc.sync.dma_start(out=out.bitcast(F32R), in_=in_.bitcast(F32R))


def build_program():
    nc = bacc.Bacc("TRN2", target_bir_lowering=False, debug=False,
                   num_devices=NCORES)

    # ---- external inputs (per-core data, same shapes on every core) ----
    def inp(name, shape):
        return nc.dram_tensor(name, list(shape), F32, kind="ExternalInput").ap()

    hT = inp("hT", (H, S))                 # hidden[b].T
    hid_own = inp("hid_own", (TOK, H))     # own-token hidden slice (natural)
    cos_qw = inp("cos_qw", (HD, S))        # cos[b].T * q_ln_w
    sin_qw = inp("sin_qw", (HD, S))        # sin[b].T * q_ln_w[perm] * sign
    cos_kw = inp("cos_kw", (HD, S))
    sin_kw = inp("sin_kw", (HD, S))
    wqkvT = inp("wqkvT", (H, 6 * P))       # [h, 4q+k+v heads] * ln1_w fold
    wdT = inp("wdT", (QH * HD, H))         # w_dense cols for this kv group, T
    gate_wT = inp("gate_wT", (H, E))       # gate_w.T * ln2_w fold
    gate_b = inp("gate_b", (1, E))
    onehot = inp("onehot", (E, 1))         # one-hot of this core's expert
    wgT = inp("wgT", (H, M))               # we_gate[e].T * ln2_w fold
    wuT = inp("wuT", (H, M))
    wdnT = inp("wdnT", (M, H))             # we_down[e].T
    wsgT = inp("wsgT", (H, M))             # shared expert, ln2_w folded
    wsuT = inp("wsuT", (H, M))
    wsdT = inp("wsdT", (M, H))

    out = nc.dram_tensor("out", [TOK, H], F32, kind="ExternalOutput").ap()

    ACT = mybir.ActivationFunctionType

    with tile.TileContext(nc) as tc:
        with (
            tc.tile_pool(name="dram", bufs=1, space="DRAM") as dram,
            tc.tile_pool(name="const", bufs=1) as const,
            tc.tile_pool(name="ps_row", bufs=2, space="PSUM") as ps_row,  # [1,512] rows
            tc.tile_pool(name="ps_tr", bufs=2, space="PSUM") as ps_tr,
            tc.tile_pool(name="ps_mm", bufs=4, space="PSUM") as ps_mm,
        ):
            # DRAM bounce buffers for collectives
            attn_in = dram.tile([S, H], F32)          # RS1 input
            attn_out_rs = dram.tile([TOK, H], F32)    # RS1 output
            ag_in = dram.tile([AGROWS, TOK], F32)     # AG input
            ag_out = dram.tile([NCORES * AGROWS, TOK], F32)
            moe_in = dram.tile([T, H], F32)           # RS2 input
            moe_out = dram.tile([TOK, H], F32)        # RS2 output
            r1_dram = dram.tile([1, S], F32)          # r1 row bounce

            cstage = const.tile([P, 1], F32)
            cstage_r = const.tile([1, P], F32)
            ones_col = const.tile([P, 1], F32)
            nc.vector.memset(cstage, 1.0)
            nc.scalar.activation(ones_col.bitcast(F32R), cstage,
                                 mybir.ActivationFunctionType.Copy)
            ones_row = const.tile([1, P], F32)
            nc.vector.memset(cstage_r, 1.0)
            nc.scalar.activation(ones_row.bitcast(F32R), cstage_r,
                                 mybir.ActivationFunctionType.Copy)
            invH_col = const.tile([P, 1], F32)
            nc.vector.memset(cstage, 1.0 / H)
            nc.scalar.activation(invH_col.bitcast(F32R), cstage,
                                 mybir.ActivationFunctionType.Copy)
            invHD_col = const.tile([P, 1], F32)
            nc.vector.memset(cstage, 1.0 / HD)
            nc.scalar.activation(invHD_col.bitcast(F32R), cstage,
                                 mybir.ActivationFunctionType.Copy)
            epsc = const.tile([P, 1], F32)
            nc.vector.memset(epsc, EPS)
            sclc = const.tile([P, 1], F32)
            nc.vector.memset(sclc, SCALE)
            ident = const.tile([P, P], F32)
            make_identity(nc, ident)

            # ============== ATTENTION (whole batch b, kv group) ==========
            with (
                tc.tile_pool(name="att_keep", bufs=1) as att_keep,
            ):
                # persistent attention-scope tiles
                qk_tiles = [att_keep.tile([P, S], F32, tag=f"qk{i}",
                                          name=f"qk{i}") for i in range(5)]
                vT = att_keep.tile([P, S], F32, tag="vT", name="vT")
                v_nat = [att_keep.tile([P, P], F32, tag=f"vn{i}",
                                       name=f"vn{i}") for i in range(SC)]
                r1 = att_keep.tile([1, S], F32, tag="r1", name="r1")
                r1T = att_keep.tile([P, SC], F32, tag="r1T", name="r1T")
                r_heads = [att_keep.tile([1, S], F32, tag=f"r_head{i}",
                                         name=f"r_head{i}")
                           for i in range(5)]

                # ---- Phase A+B: r1, qkv projections (hT resident) ----
                with (
                    tc.tile_pool(name="ab_h", bufs=HC) as ab_h,
                    tc.tile_pool(name="ab_w", bufs=3) as ab_w,
                    tc.tile_pool(name="ab_sq", bufs=2) as ab_sq,
                ):
                    h_tiles = []
                    for hc in range(HC):
                        th = ab_h.tile([P, S], F32, tag="hT")
                        _dma_r(nc, th, hT[hc * P:(hc + 1) * P, :])
                        h_tiles.append(th)

                    ps_r1 = [ps_row.tile([1, 512], F32, tag="row1",
                                          name=f"psr1_{_i}")
                             for _i in range(2)]
                    for hc in range(HC):
                        sq = ab_sq.tile([P, S], F32, tag="sq")
                        nc.vector.tensor_mul(sq.bitcast(F32R), h_tiles[hc], h_tiles[hc])
                        for t2 in range(2):
                            _mm(nc, ps_r1[t2],
                                invH_col, sq[:, t2 * 512:(t2 + 1) * 512],
                                start=(hc == 0), stop=(hc == HC - 1))
                    rms1 = ab_sq.tile([1, S], F32, tag="rms1")
                    for t2 in range(2):
                        nc.scalar.activation(rms1[:, t2 * 512:(t2 + 1) * 512],
                                             ps_r1[t2], ACT.Sqrt,
                                             bias=epsc[0:1])
                    nc.vector.reciprocal(r1, rms1)
                    # r1T[p, a] = r1[a*128 + p]  (per-chunk column form)
                    nc.sync.dma_start(out=r1_dram, in_=r1)
                    nc.sync.dma_start(
                        out=r1T,
                        in_=r1_dram.rearrange("o (a p) -> (o p) a", p=P))

                    # qkv: 6 output chunks (4q, k, v), stream weight slices
                    for oc in range(6):
                        dst = qk_tiles[oc] if oc < 5 else vT
                        for t2 in range(2):
                            pq = ps_mm.tile([P, 512], F32, tag="mm")
                            for hc in range(HC):
                                tw = ab_w.tile([P, P], F32, tag="wq")
                                _dma_r(nc, tw,
                                       wqkvT[hc * P:(hc + 1) * P,
                                             oc * P:(oc + 1) * P])
                                _mm(nc, pq, tw,
                                    h_tiles[hc][:, t2 * 512:(t2 + 1) * 512],
                                    start=(hc == 0), stop=(hc == HC - 1))
                            dslc = dst[:, t2 * 512:(t2 + 1) * 512]
                            if oc < 5:
                                dslc = dslc.bitcast(F32R)
                            nc.scalar.activation(dslc, pq, ACT.Copy)

                # v: PE-transpose to natural [s, d], scaled by r1
                for sc in range(SC):
                    pt = ps_tr.tile([P, P], F32, tag="tr")
                    nc.tensor.transpose(pt, vT[:, sc * P:(sc + 1) * P], ident)
                    nc.scalar.activation(v_nat[sc].bitcast(F32R), pt,
                                         ACT.Copy, scale=r1T[:, sc:sc + 1])

                # ---- Phase C: q_ln / k_ln rms factors (5 heads) ----
                with tc.tile_pool(name="c_tmp", bufs=2) as c_tmp:
                    for hh in range(5):
                        sq = c_tmp.tile([P, S], F32, tag="sqc")
                        nc.vector.tensor_mul(sq.bitcast(F32R), qk_tiles[hh], qk_tiles[hh])
                        rowt = c_tmp.tile([1, S], F32, tag="row_ev")
                        for t2 in range(2):
                            ps_rh = ps_row.tile([1, 512], F32, tag="row1")
                            _mm(nc, ps_rh,
                                invHD_col, sq[:, t2 * 512:(t2 + 1) * 512],
                                start=True, stop=True)
                            nc.scalar.activation(
                                rowt[:, t2 * 512:(t2 + 1) * 512],
                                ps_rh, ACT.Sqrt, bias=epsc[0:1])
                        with nc.allow_low_precision(reason="f32r round"):
                            nc.vector.reciprocal(r_heads[hh].bitcast(F32R),
                                                 rowt)
                    # fold softmax 1/sqrt(HD) into the q-head factors
                    for hh in range(4):
                        nc.scalar.activation(r_heads[hh].bitcast(F32R),
                                             r_heads[hh], ACT.Copy,
                                             scale=sclc[0:1])

                # ---- Phase D: rope (in-place into qk tiles) ----
                with (
                    tc.tile_pool(name="d_cs", bufs=1) as d_cs,
                    tc.tile_pool(name="d_tmp", bufs=2) as d_tmp,
                ):
                    cq = d_cs.tile([P, S], F32, tag="cq", name="cq")
                    sq_ = d_cs.tile([P, S], F32, tag="sq_", name="sq_")
                    ck = d_cs.tile([P, S], F32, tag="ck", name="ck")
                    sk = d_cs.tile([P, S], F32, tag="sk", name="sk")
                    nc.sync.dma_start(out=cq, in_=cos_qw[:, :])
                    nc.sync.dma_start(out=sq_, in_=sin_qw[:, :])
                    nc.sync.dma_start(out=ck, in_=cos_kw[:, :])
                    nc.sync.dma_start(out=sk, in_=sin_kw[:, :])
                    for hh in range(5):
                        cw, sw = (cq, sq_) if hh < 4 else (ck, sk)
                        src = qk_tiles[hh]
                        swp = d_tmp.tile([P, S], F32, tag="swp")
                        nc.sync.dma_start(out=swp[0:64, :], in_=src[64:128, :])
                        nc.sync.dma_start(out=swp[64:128, :], in_=src[0:64, :])
                        ta = d_tmp.tile([P, S], F32, tag="ropeA")
                        nc.vector.tensor_mul(ta, src, cw)
                        nc.vector.tensor_mul(swp, swp, sw)
                        nc.vector.tensor_add(ta, ta, swp)
                        for t2 in range(2):
                            pb = ps_mm.tile([P, 512], F32, tag="mm",
                                            name=f"pbr{hh}_{t2}")
                            _mm(nc, pb, ones_row,
                                r_heads[hh][:, t2 * 512:(t2 + 1) * 512],
                                start=True, stop=True)
                            nc.vector.tensor_mul(
                                src[:, t2 * 512:(t2 + 1) * 512].bitcast(F32R),
                                ta[:, t2 * 512:(t2 + 1) * 512], pb)

                # ---- Phase E: attention; Phase F: dense partial ----
                kT = qk_tiles[4]
                with (
                    tc.tile_pool(name="att_exp", bufs=2) as att_exp,
                    tc.tile_pool(name="att_o", bufs=1) as att_o,
                    tc.tile_pool(name="att_row", bufs=2) as att_row,
                ):
                    oT = [att_o.tile([P, S], F32, tag=f"oT{i}",
                                     name=f"oT{i}") for i in range(4)]
                    for hh in range(4):
                        qT = qk_tiles[hh]
                        for t2 in range(2):
                            ex = att_exp.tile([P, SC, 512], F32, tag="exp")
                            for sc in range(SC):
                                pst = ps_mm.tile([P, 512], F32, tag="mm")
                                _mm(nc, pst, kT[:, sc * P:(sc + 1) * P],
                                    qT[:, t2 * 512:(t2 + 1) * 512],
                                    start=True, stop=True)
                                nc.scalar.activation(
                                    ex[:, sc, :].bitcast(F32R), pst, ACT.Exp)
                            ps_den = ps_row.tile([1, 512], F32, tag="row1")
                            for sc in range(SC):
                                _mm(nc, ps_den, ones_col, ex[:, sc, :],
                                    start=(sc == 0), stop=(sc == SC - 1))
                            rden = att_row.tile([1, 512], F32, tag="rden")
                            with nc.allow_low_precision(reason="f32r round"):
                                nc.vector.reciprocal(rden.bitcast(F32R),
                                                     ps_den)
                            pb = ps_mm.tile([P, 512], F32, tag="mm")
                            _mm(nc, pb, ones_row, rden, start=True, stop=True)
                            rden_b = att_row.tile([P, 512], F32, tag="rden_b")
                            nc.scalar.activation(rden_b, pb, ACT.Copy)
                            po = ps_mm.tile([P, 512], F32, tag="mm")
                            for sc in range(SC):
                                _mm(nc, po, v_nat[sc], ex[:, sc, :],
                                    start=(sc == 0), stop=(sc == SC - 1))
                            nc.vector.tensor_mul(
                                oT[hh][:, t2 * 512:(t2 + 1) * 512]
                                .bitcast(F32R), po, rden_b)

                    # dense partial: attn_in[s, h'] = sum_o out[s,o] wd[o,h']
                    with (
                        tc.tile_pool(name="wd_pool", bufs=6) as wd_pool,
                        tc.tile_pool(name="stage", bufs=4) as stage,
                    ):
                        for nt in range(NT):
                            wd_slices = []
                            for hh in range(4):
                                twd = wd_pool.tile([P, 512], F32, tag="wd")
                                _dma_r(nc, twd,
                                       wdT[hh * P:(hh + 1) * P,
                                           nt * 512:(nt + 1) * 512])
                                wd_slices.append(twd)
                            for sc in range(SC):
                                pd = ps_mm.tile([P, 512], F32, tag="mm")
                                for hh in range(4):
                                    _mm(nc, pd,
                                        oT[hh][:, sc * P:(sc + 1) * P],
                                        wd_slices[hh],
                                        start=(hh == 0), stop=(hh == 3))
                                st = stage.tile([P, 512], F32, tag="st")
                                nc.scalar.activation(st, pd, ACT.Copy)
                                nc.sync.dma_start(
                                    out=attn_in[sc * P:(sc + 1) * P,
                                                nt * 512:(nt + 1) * 512],
                                    in_=st)

            # ============ RS1: sum partials, scatter 256-token slices ======
            nc.gpsimd.collective_compute(
                "ReduceScatter", mybir.AluOpType.add,
                replica_groups=[[0, 1, 2, 3], [4, 5, 6, 7]],
                ins=[attn_in.opt()], outs=[attn_out_rs.opt()])

            # ============ post-attention: own 256 tokens ==================
            from contextlib import ExitStack as _ES
            _keepJ_ctx = _ES()
            with (
                tc.tile_pool(name="keep2", bufs=1) as keep2,
            ):
                keepJ = _keepJ_ctx.enter_context(
                    tc.tile_pool(name="keepJ", bufs=1))
                hidden2 = [keepJ.tile([P, H], F32, tag=f"h2_{i}",
                                      name=f"h2_{i}") for i in range(TC)]
                x2T_own = [keepJ.tile([P, TOK], F32, tag=f"x2T{i}",
                                      name=f"x2T{i}") for i in range(HC)]
                resid2 = [keep2.tile([P, H], F32, tag=f"res{i}",
                                     name=f"res{i}") for i in range(TC)]

                # ---- Phase G: residual + ln2 + transpose ----
                with tc.tile_pool(name="g_tmp", bufs=2) as g_tmp:
                    for tcc in range(TC):
                        rs = g_tmp.tile([P, H], F32, tag="rs")
                        nc.sync.dma_start(
                            out=rs, in_=attn_out_rs[tcc * P:(tcc + 1) * P, :])
                        ho = g_tmp.tile([P, H], F32, tag="ho")
                        nc.sync.dma_start(
                            out=ho, in_=hid_own[tcc * P:(tcc + 1) * P, :])
                        nc.vector.tensor_add(hidden2[tcc], rs, ho)
                        sq = g_tmp.tile([P, H], F32, tag="sqg")
                        nc.vector.tensor_mul(sq, hidden2[tcc], hidden2[tcc])
                        ssum = g_tmp.tile([P, 1], F32, tag="ssum")
                        nc.vector.reduce_sum(ssum, sq,
                                             axis=mybir.AxisListType.X)
                        rms2 = g_tmp.tile([P, 1], F32, tag="rms2")
                        nc.scalar.activation(rms2, ssum, ACT.Sqrt,
                                             bias=epsc, scale=invH_col)
                        r2 = g_tmp.tile([P, 1], F32, tag="r2")
                        nc.vector.reciprocal(r2, rms2)
                        x2 = g_tmp.tile([P, H], F32, tag="x2")
                        nc.scalar.activation(x2, hidden2[tcc], ACT.Copy,
                                             scale=r2)
                        for hc in range(HC):
                            pt = ps_tr.tile([P, P], F32, tag="tr")
                            nc.tensor.transpose(
                                pt, x2[:, hc * P:(hc + 1) * P], ident)
                            nc.scalar.activation(
                                x2T_own[hc][:, tcc * P:(tcc + 1) * P]
                                .bitcast(F32R), pt, ACT.Copy)
                    for hc in range(HC):
                        nc.sync.dma_start(out=ag_in[hc * P:(hc + 1) * P, :],
                                          in_=x2T_own[hc])

                # ---- Phase H: routing on own tokens ----
                with tc.tile_pool(name="r_tmp", bufs=2) as r_tmp:
                    gw = r_tmp.tile([P, HC, E], F32, tag="gw")
                    nc.sync.dma_start(
                        out=gw.bitcast(F32R),
                        in_=gate_wT.rearrange("(c p) e -> p c e",
                                              p=P).bitcast(F32R))
                    gb = r_tmp.tile([P, E], F32, tag="gb")
                    nc.sync.dma_start(out=gb, in_=gate_b.to_broadcast((P, E)))
                    for tcc in range(TC):
                        pl = ps_tr.tile([P, E], F32, tag="tr")
                        for hc in range(HC):
                            _mm(nc, pl, x2T_own[hc][:, tcc * P:(tcc + 1) * P],
                                gw[:, hc, :], start=(hc == 0),
                                stop=(hc == HC - 1))
                        ssig = r_tmp.tile([P, E], F32, tag="ssig")
                        nc.scalar.activation(ssig, pl, ACT.Sigmoid)
                        sb = r_tmp.tile([P, E], F32, tag="sbt")
                        nc.vector.tensor_add(sb, ssig, gb)
                        sbg = sb.rearrange("p (g two) -> p g two", two=2)
                        g4 = r_tmp.tile([P, NG], F32, tag="g4")
                        nc.vector.tensor_add(g4, sbg[:, :, 0], sbg[:, :, 1])
                        m1 = r_tmp.tile([P, 1], F32, tag="m1")
                        nc.vector.reduce_max(m1, g4, axis=mybir.AxisListType.X)
                        eq1 = r_tmp.tile([P, NG], F32, tag="eq1")
                        nc.vector.tensor_scalar(eq1, g4, m1, -BIG,
                                                mybir.AluOpType.is_equal,
                                                mybir.AluOpType.mult)
                        g4b = r_tmp.tile([P, NG], F32, tag="g4b")
                        nc.vector.tensor_add(g4b, g4, eq1)
                        m2 = r_tmp.tile([P, 1], F32, tag="m2")
                        nc.vector.reduce_max(m2, g4b,
                                             axis=mybir.AxisListType.X)
                        gmask = r_tmp.tile([P, NG], F32, tag="gmask")
                        nc.vector.tensor_scalar(gmask, g4, m2, None,
                                                mybir.AluOpType.is_ge)
                        pen = r_tmp.tile([P, E], F32, tag="pen")
                        peng = pen.rearrange("p (g two) -> p g two", two=2)
                        nc.vector.tensor_scalar(peng[:, :, 0], gmask, BIG,
                                                -BIG, mybir.AluOpType.mult,
                                                mybir.AluOpType.add)
                        nc.vector.tensor_scalar(peng[:, :, 1], gmask, BIG,
                                                -BIG, mybir.AluOpType.mult,
                                                mybir.AluOpType.add)
                        ms = r_tmp.tile([P, E], F32, tag="ms")
                        nc.vector.tensor_add(ms, sb, pen)
                        t1 = r_tmp.tile([P, 1], F32, tag="t1")
                        nc.vector.reduce_max(t1, ms, axis=mybir.AxisListType.X)
                        eq2 = r_tmp.tile([P, E], F32, tag="eq2")
                        nc.vector.tensor_scalar(eq2, ms, t1, -BIG,
                                                mybir.AluOpType.is_equal,
                                                mybir.AluOpType.mult)
                        ms2 = r_tmp.tile([P, E], F32, tag="ms2")
                        nc.vector.tensor_add(ms2, ms, eq2)
                        t2r = r_tmp.tile([P, 1], F32, tag="t2r")
                        nc.vector.reduce_max(t2r, ms2,
                                             axis=mybir.AxisListType.X)
                        sel = r_tmp.tile([P, E], F32, tag="sel")
                        nc.vector.tensor_scalar(sel, ms, t2r, None,
                                                mybir.AluOpType.is_ge)
                        wsel = r_tmp.tile([P, E], F32, tag="wsel")
                        nc.vector.tensor_mul(wsel, ssig, sel)
                        dsum = r_tmp.tile([P, 1], F32, tag="dsum")
                        nc.vector.reduce_sum(dsum, wsel,
                                             axis=mybir.AxisListType.X)
                        nc.vector.tensor_scalar_add(dsum, dsum, 1e-20)
                        rdsum = r_tmp.tile([P, 1], F32, tag="rdsum")
                        nc.vector.reciprocal(rdsum, dsum)
                        route = r_tmp.tile([P, E], F32, tag="route")
                        nc.vector.tensor_scalar(route, wsel, rdsum, None,
                                                mybir.AluOpType.mult)
                        ptr = ps_tr.tile([E, P], F32, tag="tr")
                        nc.tensor.transpose(ptr, route, ident)
                        rT = r_tmp.tile([E, P], F32, tag="rTst")
                        nc.scalar.activation(rT, ptr, ACT.Copy)
                        nc.sync.dma_start(
                            out=ag_in[H:H + E, tcc * P:(tcc + 1) * P], in_=rT)

                # ===== AllGather x2T + routeT across all 8 cores =====
                nc.gpsimd.collective_compute(
                    "AllGather", mybir.AluOpType.bypass,
                    replica_groups=[list(range(NCORES))],
                    ins=[ag_in.opt()], outs=[ag_out.opt()])

                # ---- Phase J: shared expert on own tokens -> resid2 ----
                with (
                    tc.tile_pool(name="sh_w", bufs=3) as sh_w,
                    tc.tile_pool(name="sh_hs", bufs=1) as sh_hs,
                    tc.tile_pool(name="sh_tmp", bufs=2) as sh_tmp,
                ):
                    hs_t = [sh_hs.tile([P, TOK], F32, tag=f"hs{i}",
                                       name=f"hs{i}") for i in range(MC)]
                    for mc in range(MC):
                        pg = ps_mm.tile([P, TOK], F32, tag="mm")
                        for hc in range(HC):
                            tw = sh_w.tile([P, P], F32, tag="wsg")
                            _dma_r(nc, tw, wsgT[hc * P:(hc + 1) * P,
                                                mc * P:(mc + 1) * P])
                            _mm(nc, pg, tw, x2T_own[hc], start=(hc == 0),
                                stop=(hc == HC - 1))
                        sg = sh_tmp.tile([P, TOK], F32, tag="sg")
                        nc.scalar.activation(sg, pg, ACT.Silu)
                        pu = ps_mm.tile([P, TOK], F32, tag="mm")
                        for hc in range(HC):
                            tw = sh_w.tile([P, P], F32, tag="wsu")
                            _dma_r(nc, tw, wsuT[hc * P:(hc + 1) * P,
                                                mc * P:(mc + 1) * P])
                            _mm(nc, pu, tw, x2T_own[hc], start=(hc == 0),
                                stop=(hc == HC - 1))
                        nc.vector.tensor_mul(hs_t[mc].bitcast(F32R), pu, sg)
                    for nt in range(NT):
                        pds = [ps_mm.tile([P, 512], F32, tag="mm",
                                          name=f"pds{_i}")
                               for _i in range(TC)]
                        for mc in range(MC):
                            tw = sh_w.tile([P, 512], F32, tag="wsd")
                            _dma_r(nc, tw, wsdT[mc * P:(mc + 1) * P,
                                                nt * 512:(nt + 1) * 512])
                            for tcc in range(TC):
                                _mm(nc, pds[tcc],
                                    hs_t[mc][:, tcc * P:(tcc + 1) * P], tw,
                                    start=(mc == 0), stop=(mc == MC - 1))
                        for tcc in range(TC):
                            nc.vector.tensor_add(
                                resid2[tcc][:, nt * 512:(nt + 1) * 512],
                                pds[tcc],
                                hidden2[tcc][:, nt * 512:(nt + 1) * 512])

                _keepJ_ctx.close()

                # ---- Phase K: this core's expert over all 2048 tokens ----
                with (
                    tc.tile_pool(name="moe_w", bufs=1) as moe_w,
                    tc.tile_pool(name="moe_xb", bufs=HC + 2) as moe_xb,
                    tc.tile_pool(name="moe_rt", bufs=2) as moe_rt,
                    tc.tile_pool(name="moe_h", bufs=2) as moe_h,
                    tc.tile_pool(name="moe_st", bufs=4) as moe_st,
                ):
                    wg_t, wu_t, wdn_t = [], [], []
                    for hc in range(HC):
                        twg = moe_w.tile([P, M], F32, tag=f"wg{hc}",
                                         name=f"wg{hc}")
                        _dma_r(nc, twg, wgT[hc * P:(hc + 1) * P, :])
                        wg_t.append(twg)
                        twu = moe_w.tile([P, M], F32, tag=f"wu{hc}",
                                         name=f"wu{hc}")
                        _dma_r(nc, twu, wuT[hc * P:(hc + 1) * P, :])
                        wu_t.append(twu)
                    for mc in range(MC):
                        twd = moe_w.tile([P, H], F32, tag=f"wdn{mc}",
                                         name=f"wdn{mc}")
                        _dma_r(nc, twd, wdnT[mc * P:(mc + 1) * P, :])
                        wdn_t.append(twd)
                    oh = moe_w.tile([E, 1], F32, tag="oh", name="oh")
                    _dma_r(nc, oh, onehot[:, :])

                    for tb in range(TBLK):
                        base = tb * AGROWS
                        xb = []
                        for hc in range(HC):
                            tx = moe_xb.tile([P, TOK], F32, tag="xb")
                            _dma_r(nc, tx,
                                   ag_out[base + hc * P:base + (hc + 1) * P, :])
                            xb.append(tx)
                        rts = moe_rt.tile([E, TOK], F32, tag="rts")
                        _dma_r(nc, rts, ag_out[base + H:base + H + E, :])
                        prr = ps_tr.tile([1, TOK], F32, tag="tr")
                        _mm(nc, prr, oh, rts, start=True, stop=True)
                        rblk = moe_rt.tile([1, TOK], F32, tag="rblk")
                        nc.scalar.activation(rblk.bitcast(F32R), prr, ACT.Copy)
                        pbb = ps_tr.tile([P, TOK], F32, tag="tr")
                        _mm(nc, pbb, ones_row, rblk, start=True, stop=True)
                        rblk_b = moe_rt.tile([P, TOK], F32, tag="rblk_b")
                        nc.scalar.activation(rblk_b, pbb, ACT.Copy)

                        h_t = []
                        for mc in range(MC):
                            pg = ps_mm.tile([P, TOK], F32, tag="mm")
                            for hc in range(HC):
                                _mm(nc, pg, wg_t[hc][:, mc * P:(mc + 1) * P],
                                    xb[hc], start=(hc == 0),
                                    stop=(hc == HC - 1))
                            sg = moe_h.tile([P, TOK], F32, tag="msg")
                            nc.scalar.activation(sg, pg, ACT.Silu)
                            pu = ps_mm.tile([P, TOK], F32, tag="mm")
                            for hc in range(HC):
                                _mm(nc, pu, wu_t[hc][:, mc * P:(mc + 1) * P],
                                    xb[hc], start=(hc == 0),
                                    stop=(hc == HC - 1))
                            ur = moe_h.tile([P, TOK], F32, tag="mur")
                            nc.vector.tensor_mul(ur, pu, rblk_b)
                            ht = moe_h.tile([P, TOK], F32, tag=f"mh{mc}")
                            nc.vector.tensor_mul(ht.bitcast(F32R), ur, sg)
                            h_t.append(ht)

                        for tcc in range(TC):
                            for nt in range(NT):
                                pd = ps_mm.tile([P, 512], F32, tag="mm")
                                for mc in range(MC):
                                    _mm(nc, pd,
                                        h_t[mc][:, tcc * P:(tcc + 1) * P],
                                        wdn_t[mc][:, nt * 512:(nt + 1) * 512],
                                        start=(mc == 0), stop=(mc == MC - 1))
                                st = moe_st.tile([P, 512], F32, tag="mst")
                                nc.scalar.activation(st, pd, ACT.Copy)
                                nc.sync.dma_start(
                                    out=moe_in[
                                        tb * TOK + tcc * P:
                                        tb * TOK + (tcc + 1) * P,
                                        nt * 512:(nt + 1) * 512],
                                    in_=st)

                # ===== RS2 + final residual add =====
                nc.gpsimd.collective_compute(
                    "ReduceScatter", mybir.AluOpType.add,
                    replica_groups=[list(range(NCORES))],
                    ins=[moe_in.opt()], outs=[moe_out.opt()])

                with tc.tile_pool(name="fin", bufs=4) as fin:
                    for tcc in range(TC):
                        mo = fin.tile([P, H], F32, tag="mo")
                        nc.sync.dma_start(
                            out=mo, in_=moe_out[tcc * P:(tcc + 1) * P, :])
                        fo = fin.tile([P, H], F32, tag="fo")
                        nc.vector.tensor_add(fo, mo, resid2[tcc])
                        nc.sync.dma_start(
                            out=out[tcc * P:(tcc + 1) * P, :], in_=fo)

    nc.compile()
    return nc


def prep_inputs(c, inputs):
    """Build the per-core input map (all host-side layout/weight-fold work)."""
    f = np.float32
    hs = np.asarray(inputs["hidden_states"], f)
    cos = np.asarray(inputs["cos"], f)
    sin = np.asarray(inputs["sin"], f)
    ln1 = np.asarray(inputs["ln1_w"], f)
    w_qkv = np.asarray(inputs["w_qkv"], f)
    q_ln = np.asarray(inputs["q_ln_w"], f)
    k_ln = np.asarray(inputs["k_ln_w"], f)
    w_dense = np.asarray(inputs["w_dense"], f)
    ln2 = np.asarray(inputs["ln2_w"], f)
    gate_w = np.asarray(inputs["gate_w"], f)
    gate_b = np.asarray(inputs["gate_bias"], f)
    we_gate = np.asarray(inputs["we_gate"], f)
    we_up = np.asarray(inputs["we_up"], f)
    we_down = np.asarray(inputs["we_down"], f)
    ws_gate = np.asarray(inputs["ws_gate"], f)
    ws_up = np.asarray(inputs["ws_up"], f)
    ws_down = np.asarray(inputs["ws_down"], f)

    b, kv, e = c // 4, c % 4, c
    perm = np.concatenate([np.arange(64, 128), np.arange(64)])
    sign = np.concatenate([-np.ones(64, f), np.ones(64, f)])

    q_rows = np.arange(kv * QH * HD, (kv + 1) * QH * HD)
    k_rows = np.arange(NH * HD + kv * HD, NH * HD + (kv + 1) * HD)
    v_rows = np.arange(NH * HD + NKV * HD + kv * HD,
                       NH * HD + NKV * HD + (kv + 1) * HD)
    sel = np.concatenate([q_rows, k_rows, v_rows])
    w_sel = w_qkv[sel] * ln1[None, :]

    onehot = np.zeros((E, 1), f)
    onehot[e, 0] = 1.0

    C = np.ascontiguousarray
    return {
        "hT": C(hs[b].T),
        "hid_own": C(hs[b, kv * TOK:(kv + 1) * TOK]),
        "cos_qw": C(cos[b].T * q_ln[:, None]),
        "sin_qw": C(sin[b].T * (q_ln[perm] * sign)[:, None]),
        "cos_kw": C(cos[b].T * k_ln[:, None]),
        "sin_kw": C(sin[b].T * (k_ln[perm] * sign)[:, None]),
        "wqkvT": C(w_sel.T),
        "wdT": C(w_dense[:, kv * QH * HD:(kv + 1) * QH * HD].T),
        "gate_wT": C((gate_w * ln2[None, :]).T),
        "gate_b": C(gate_b.reshape(1, E)),
        "onehot": onehot,
        "wgT": C((we_gate[e] * ln2[None, :]).T),
        "wuT": C((we_up[e] * ln2[None, :]).T),
        "wdnT": C(we_down[e].T),
        "wsgT": C((ws_gate * ln2[None, :]).T),
        "wsuT": C((ws_up * ln2[None, :]).T),
        "wsdT": C(ws_down.T),
    }


_NC_CACHE = {}


def get_program():
    if "nc" not in _NC_CACHE:
        _NC_CACHE["nc"] = build_program()
    return _NC_CACHE["nc"]


def kernel(**inputs) -> np.ndarray:
    from concourse.bass_utils import run_bass_kernel_spmd

    nc = get_program()
    in_maps = [prep_inputs(c, inputs) for c in range(NCORES)]
    res = run_bass_kernel_spmd(nc, in_maps, core_ids=list(range(NCORES)))
    slices = [res.results[c]["out"] for c in range(NCORES)]
    full = np.concatenate(slices, axis=0)          # [2048, 2048] token-major
    return full.reshape(B, S, H).astype(np.float32)



# revision 50
# speedup vs baseline: 1.7568x; 1.7568x over previous
"""LLaDA2 MoE decoder layer as a single SPMD Bass/Tile kernel on 8 TRN2 cores.

Sharding:
  - Attention: core c handles batch b = c//4 and kv-group kv = c%4
    (1 kv head + its 4 q heads, full 1024-token sequence of batch b).
    Two 4-core ReduceScatters (bf16, interleaved row order) sum the w_dense
    partial products and hand each core its 256-token slice in two
    128-token chunks, the first overlapping the dense projection tail.
  - MoE: expert-parallel. Each core owns expert e = c. x2 (post-ln2
    activations, transposed, bf16) + routing weights are AllGathered
    (overlapped with the shared expert) so every core runs its expert over
    all 2048 tokens. The down-projection emits h'-column chunks; four
    column-wise ReduceScatters (bf16) pipeline behind the compute and
    deliver each core its own 256-token rows, so the final
    residual + shared-expert add stays local.

Routing decisions are computed in f32r (top-k flips are the one thing bf16
could corrupt); bulk GEMM payloads ride in bf16 with fp32 PSUM accumulation.
"""

import math
import os

import numpy as np
import ml_dtypes

import concourse.bass as bass
import concourse.mybir as mybir
import concourse.tile as tile
from concourse import bacc
from concourse.masks import make_identity

# Problem shapes (hardcoded per contest rules).
B, S, H = 2, 1024, 2048
NH, NKV, HD = 16, 4, 128
E, TOPK, NG, TOPKG, M = 8, 2, 4, 2, 512
EPS = 1e-6
SCALE = HD ** -0.5
T = B * S                      # 2048 tokens
NCORES = 8
TOK = T // NCORES              # 256 tokens owned per core
QH = NH // NKV                 # 4 q heads per kv head
P = 128
F32 = mybir.dt.float32
F32R = mybir.dt.float32r
BF16 = mybir.dt.bfloat16
BIG = 1e30

HC = H // P                    # 16 h chunks
SC = S // P                    # 8 seq chunks per batch
MC = M // P                    # 4 m chunks
TC = TOK // P                  # 2 own-token chunks
NT = H // 512                  # 4 512-wide h' tiles
TBLK = NCORES                  # 8 token blocks of 256 after AG
TQ = T // 512                  # 4 512-token quarters
AGCOLS = HC * TOK + TC * E     # 4112 cols in partition-major AG block


def r32(ap):
    return ap.bitcast(F32R)


def _mm(nc, out, lhsT, rhs, start, stop):
    nc.tensor.matmul(out, r32(lhsT), r32(rhs), start=start, stop=stop)


def _dma_r(nc, out, in_):
    nc.sync.dma_start(out=out.bitcast(F32R), in_=in_.bitcast(F32R))


def build_program():
    nc = bacc.Bacc("TRN2", target_bir_lowering=False, debug=False,
                   num_devices=NCORES)

    # ---- external inputs (per-core data, same shapes on every core) ----
    def inp(name, shape, dt=F32):
        return nc.dram_tensor(name, list(shape), dt, kind="ExternalInput").ap()

    hT = inp("hT", (H, S))                 # hidden[b].T
    hid_own = inp("hid_own", (TOK, H))     # own-token hidden slice (natural)
    cos_qw = inp("cos_qw", (HD, S))        # cos[b].T * q_ln_w
    sin_qw = inp("sin_qw", (HD, S))        # sin[b].T * q_ln_w[perm] * sign
    cos_kw = inp("cos_kw", (HD, S))
    sin_kw = inp("sin_kw", (HD, S))
    wqkvT = inp("wqkvT", (H, 6 * P))       # [h, 4q+k+v heads] * ln1_w fold
    wdT = inp("wdT", (QH * HD, H))         # w_dense cols for this kv group, T
    gate_wT = inp("gate_wT", (H, E))       # gate_w.T * ln2_w fold
    gate_b = inp("gate_b", (1, E))
    oh_row = inp("oh_row", (1, E))         # one-hot of this core's expert
    wgT = inp("wgT", (H, M), BF16)         # we_gate[e].T * ln2_w fold, bf16
    wuT = inp("wuT", (H, M), BF16)
    wdnT = inp("wdnT", (M, H), BF16)       # we_down[e].T, bf16
    wsgT = inp("wsgT", (H, M), BF16)       # shared expert, ln2_w folded, bf16
    wsuT = inp("wsuT", (H, M), BF16)
    wsdT = inp("wsdT", (M, H), BF16)

    out = nc.dram_tensor("out", [TOK, H], F32, kind="ExternalOutput").ap()

    ACT = mybir.ActivationFunctionType

    with tile.TileContext(nc) as tc:
        with (
            tc.tile_pool(name="dram", bufs=1, space="DRAM") as dram,
            tc.tile_pool(name="const", bufs=1) as const,
        ):
            # DRAM bounce buffers for collectives (outputs Shared)
            rs1_in = [dram.tile([4 * P, H], BF16, tag=f"rs1i{i}",
                                name=f"rs1i{i}") for i in range(2)]
            rs1_out = [dram.tile([P, H], BF16, tag=f"rs1o{i}",
                                 name=f"rs1o{i}") for i in range(2)]
            # partition-major AG payload: per core one [128, AGCOLS] block
            # (cols = 16 x2T chunks of 256 tokens, then 2x8 natural routes)
            ag_in = dram.tile([P, AGCOLS], BF16)
            ag_out = dram.tile([NCORES * P, AGCOLS], BF16,
                               addr_space="Shared")
            rs2_in = [dram.tile([T, 512], BF16, tag=f"rs2i{i}",
                                name=f"rs2i{i}") for i in range(NT)]
            rs2_out = [dram.tile([TOK, 512], BF16, tag=f"rs2o{i}",
                                 name=f"rs2o{i}") for i in range(NT)]
            r1_dram = dram.tile([1, S], F32)          # r1 row bounce
            rt_dram = dram.tile([1, T], F32)          # route row bounce

            cstage = const.tile([P, 1], F32)
            cstage_r = const.tile([1, P], F32)
            ones_col = const.tile([P, 1], F32)
            nc.vector.memset(cstage, 1.0)
            nc.scalar.activation(ones_col.bitcast(F32R), cstage, ACT.Copy)
            ones_row = const.tile([1, P], F32)
            nc.vector.memset(cstage_r, 1.0)
            nc.scalar.activation(ones_row.bitcast(F32R), cstage_r, ACT.Copy)
            invH_col = const.tile([P, 1], F32)
            nc.vector.memset(cstage, 1.0 / H)
            nc.scalar.activation(invH_col.bitcast(F32R), cstage, ACT.Copy)
            invHD_col = const.tile([P, 1], F32)
            nc.vector.memset(cstage, 1.0 / HD)
            nc.scalar.activation(invHD_col.bitcast(F32R), cstage, ACT.Copy)
            epsc = const.tile([P, 1], F32)
            nc.vector.memset(epsc, EPS)
            sclc = const.tile([P, 1], F32)
            nc.vector.memset(sclc, SCALE)
            ident = const.tile([P, P], F32)
            make_identity(nc, ident)

            # ============== ATTENTION (whole batch b, kv group) ==========
            with (
                tc.tile_pool(name="att_keep", bufs=1) as att_keep,
                tc.tile_pool(name="ps_row", bufs=2, space="PSUM") as ps_row,
                tc.tile_pool(name="ps_mm", bufs=6, space="PSUM") as ps_mm,
            ):
                # persistent attention-scope tiles
                qk_tiles = [att_keep.tile([P, S], F32, tag=f"qk{i}",
                                          name=f"qk{i}") for i in range(5)]
                vT = att_keep.tile([P, S], F32, tag="vT", name="vT")
                v_nat = [att_keep.tile([P, P], F32, tag=f"vn{i}",
                                       name=f"vn{i}") for i in range(SC)]
                r1 = att_keep.tile([1, S], F32, tag="r1", name="r1")
                r1T = att_keep.tile([P, SC], F32, tag="r1T", name="r1T")
                r_heads = [att_keep.tile([1, S], F32, tag=f"r_head{i}",
                                         name=f"r_head{i}")
                           for i in range(5)]

                # ---- Phase A+B: r1, qkv projections (hT + wqkv resident) --
                with (
                    tc.tile_pool(name="ab_h", bufs=1) as ab_h,
                    tc.tile_pool(name="ab_w", bufs=1) as ab_w,
                    tc.tile_pool(name="ab_sq", bufs=2) as ab_sq,
                ):
                    h_big = ab_h.tile([P, HC, S], F32, tag="hT", name="hbig")
                    hT_r = hT.rearrange("(c p) s -> p c s", p=P)
                    for g in range(4):
                        nc.sync.dma_start(
                            out=h_big[:, g * 4:(g + 1) * 4, :].bitcast(F32R),
                            in_=hT_r[:, g * 4:(g + 1) * 4, :].bitcast(F32R))
                    w_big = ab_w.tile([P, HC, 6 * P], F32, tag="wq",
                                      name="wbig")
                    wq_r = wqkvT.rearrange("(c p) o -> p c o", p=P)
                    for g in range(4):
                        nc.sync.dma_start(
                            out=w_big[:, g * 4:(g + 1) * 4, :].bitcast(F32R),
                            in_=wq_r[:, g * 4:(g + 1) * 4, :].bitcast(F32R))
                    h_tiles = [h_big[:, hc, :] for hc in range(HC)]
                    w_tiles = [w_big[:, hc, :] for hc in range(HC)]

                    ps_r1 = [ps_row.tile([1, 512], F32, tag="row1",
                                          name=f"psr1_{_i}")
                             for _i in range(2)]
                    for hc in range(HC):
                        sq = ab_sq.tile([P, S], F32, tag="sq")
                        nc.vector.tensor_mul(sq.bitcast(F32R), h_tiles[hc],
                                             h_tiles[hc])
                        for t2 in range(2):
                            _mm(nc, ps_r1[t2],
                                invH_col, sq[:, t2 * 512:(t2 + 1) * 512],
                                start=(hc == 0), stop=(hc == HC - 1))
                    rms1 = ab_sq.tile([1, S], F32, tag="rms1")
                    for t2 in range(2):
                        nc.scalar.activation(rms1[:, t2 * 512:(t2 + 1) * 512],
                                             ps_r1[t2], ACT.Sqrt,
                                             bias=epsc[0:1])
                    nc.vector.reciprocal(r1, rms1)
                    # r1T[p, a] = r1[a*128 + p]  (per-chunk column form)
                    nc.sync.dma_start(out=r1_dram, in_=r1)
                    nc.sync.dma_start(
                        out=r1T,
                        in_=r1_dram.rearrange("o (a p) -> (o p) a", p=P))

                    # qkv: 6 output chunks (4q, k, v)
                    for oc in range(6):
                        dst = qk_tiles[oc] if oc < 5 else vT
                        for t2 in range(2):
                            pq = ps_mm.tile([P, 512], F32, tag="mm")
                            for hc in range(HC):
                                _mm(nc, pq,
                                    w_tiles[hc][:, oc * P:(oc + 1) * P],
                                    h_tiles[hc][:, t2 * 512:(t2 + 1) * 512],
                                    start=(hc == 0), stop=(hc == HC - 1))
                            dslc = dst[:, t2 * 512:(t2 + 1) * 512]
                            if oc < 5:
                                dslc = dslc.bitcast(F32R)
                            nc.scalar.activation(dslc, pq, ACT.Copy)

                # v: PE-transpose to natural [s, d], scaled by r1
                for sc in range(SC):
                    ptw = ps_mm.tile([P, 512], F32, tag="mm")
                    pt = ptw[:, 0:P]
                    nc.tensor.transpose(pt, vT[:, sc * P:(sc + 1) * P], ident)
                    nc.scalar.activation(v_nat[sc].bitcast(F32R), pt,
                                         ACT.Copy, scale=r1T[:, sc:sc + 1])

                # ---- Phases C+D+E+F fused, per-head pipelined ----
                # Head prep (rms factor + rope) for head hh runs while the
                # previous head's scores/AV matmuls occupy the PE; token-half
                # major so RS1 chunk 0 hides behind the second half.
                kT = qk_tiles[4]
                with (
                    tc.tile_pool(name="cd_tmp", bufs=2) as cd_tmp,
                    tc.tile_pool(name="d_cs", bufs=1) as d_cs,
                    tc.tile_pool(name="att_exp", bufs=2) as att_exp,
                    tc.tile_pool(name="att_o", bufs=1) as att_o,
                    tc.tile_pool(name="att_row", bufs=3) as att_row,
                    tc.tile_pool(name="wd_pool", bufs=1) as wd_pool,
                    tc.tile_pool(name="stage", bufs=3) as stage,
                ):
                    cq = d_cs.tile([P, S], F32, tag="cq", name="cq")
                    sq_ = d_cs.tile([P, S], F32, tag="sq_", name="sq_")
                    ck = d_cs.tile([P, S], F32, tag="ck", name="ck")
                    sk = d_cs.tile([P, S], F32, tag="sk", name="sk")
                    nc.sync.dma_start(out=cq, in_=cos_qw[:, :])
                    nc.sync.dma_start(out=sq_, in_=sin_qw[:, :])
                    nc.sync.dma_start(out=ck, in_=cos_kw[:, :])
                    nc.sync.dma_start(out=sk, in_=sin_kw[:, :])
                    wd_big = wd_pool.tile([P, QH, H], F32, tag="wd",
                                          name="wdbig")
                    nc.sync.dma_start(
                        out=wd_big.bitcast(F32R),
                        in_=wdT.rearrange("(c p) o -> p c o",
                                          p=P).bitcast(F32R))
                    wd_slices = [wd_big[:, hh, :] for hh in range(QH)]

                    def head_prep(hh):
                        # C: rms factor for this head
                        sq = cd_tmp.tile([P, S], F32, tag="sqc")
                        nc.vector.tensor_mul(sq.bitcast(F32R), qk_tiles[hh],
                                             qk_tiles[hh])
                        rowt = cd_tmp.tile([1, S], F32, tag="row_ev")
                        for t2 in range(2):
                            ps_rh = ps_row.tile([1, 512], F32, tag="row1")
                            _mm(nc, ps_rh,
                                invHD_col, sq[:, t2 * 512:(t2 + 1) * 512],
                                start=True, stop=True)
                            nc.scalar.activation(
                                rowt[:, t2 * 512:(t2 + 1) * 512],
                                ps_rh, ACT.Sqrt, bias=epsc[0:1])
                        with nc.allow_low_precision(reason="f32r round"):
                            nc.vector.reciprocal(r_heads[hh].bitcast(F32R),
                                                 rowt)
                        if hh < 4:
                            # fold softmax 1/sqrt(HD) into q-head factors
                            nc.scalar.activation(r_heads[hh].bitcast(F32R),
                                                 r_heads[hh], ACT.Copy,
                                                 scale=sclc[0:1])
                        # D: rope in place
                        cw, sw = (cq, sq_) if hh < 4 else (ck, sk)
                        src = qk_tiles[hh]
                        swp = cd_tmp.tile([P, S], F32, tag="swp")
                        nc.sync.dma_start(out=swp[0:64, :], in_=src[64:128, :])
                        nc.sync.dma_start(out=swp[64:128, :], in_=src[0:64, :])
                        ta = cd_tmp.tile([P, S], F32, tag="ropeA")
                        nc.vector.tensor_mul(ta, src, cw)
                        nc.vector.tensor_mul(swp, swp, sw)
                        nc.vector.tensor_add(ta, ta, swp)
                        for t2 in range(2):
                            pb = ps_mm.tile([P, 512], F32, tag="mm")
                            _mm(nc, pb, ones_row,
                                r_heads[hh][:, t2 * 512:(t2 + 1) * 512],
                                start=True, stop=True)
                            nc.vector.tensor_mul(
                                src[:, t2 * 512:(t2 + 1) * 512].bitcast(F32R),
                                ta[:, t2 * 512:(t2 + 1) * 512], pb)

                    oT = [att_o.tile([P, S], F32, tag=f"oT{i}",
                                     name=f"oT{i}") for i in range(4)]
                    for t2 in range(2):
                        for hh in range(4):
                            if t2 == 0:
                                if hh == 0:
                                    head_prep(4)
                                head_prep(hh)
                            qT = qk_tiles[hh]
                            ex = att_exp.tile([P, SC, 512], F32, tag="exp")
                            for sc in range(SC):
                                pst = ps_mm.tile([P, 512], F32, tag="mm")
                                _mm(nc, pst, kT[:, sc * P:(sc + 1) * P],
                                    qT[:, t2 * 512:(t2 + 1) * 512],
                                    start=True, stop=True)
                                nc.scalar.activation(
                                    ex[:, sc, :].bitcast(F32R), pst, ACT.Exp)
                            ps_den = ps_row.tile([1, 512], F32, tag="row1")
                            for sc in range(SC):
                                _mm(nc, ps_den, ones_col, ex[:, sc, :],
                                    start=(sc == 0), stop=(sc == SC - 1))
                            rden = att_row.tile([1, 512], F32, tag="rden")
                            with nc.allow_low_precision(reason="f32r round"):
                                nc.vector.reciprocal(rden.bitcast(F32R),
                                                     ps_den)
                            pb = ps_mm.tile([P, 512], F32, tag="mm")
                            _mm(nc, pb, ones_row, rden, start=True, stop=True)
                            rden_b = att_row.tile([P, 512], F32, tag="rden_b")
                            nc.scalar.activation(rden_b, pb, ACT.Copy)
                            po = ps_mm.tile([P, 512], F32, tag="mm")
                            for sc in range(SC):
                                _mm(nc, po, v_nat[sc], ex[:, sc, :],
                                    start=(sc == 0), stop=(sc == SC - 1))
                            nc.vector.tensor_mul(
                                oT[hh][:, t2 * 512:(t2 + 1) * 512]
                                .bitcast(F32R), po, rden_b)

                        # dense partial for this token half, then RS1 chunk.
                        # one [P, H] staging tile and a single DMA per sc
                        # keeps the sync queue quiet while collectives fly.
                        j = t2
                        for scx in range(4):
                            sc = 4 * j + scx
                            st = stage.tile([P, H], BF16, tag="st")
                            for nt in range(NT):
                                pd = ps_mm.tile([P, 512], F32, tag="mm")
                                for hh in range(4):
                                    _mm(nc, pd,
                                        oT[hh][:, sc * P:(sc + 1) * P],
                                        wd_slices[hh][:,
                                                      nt * 512:(nt + 1) * 512],
                                        start=(hh == 0), stop=(hh == 3))
                                nc.scalar.activation(
                                    st[:, nt * 512:(nt + 1) * 512], pd,
                                    ACT.Copy)
                            nc.sync.dma_start(
                                out=rs1_in[j][scx * P:(scx + 1) * P, :],
                                in_=st)
                        nc.gpsimd.collective_compute(
                            "ReduceScatter", mybir.AluOpType.add,
                            replica_groups=[[0, 1, 2, 3], [4, 5, 6, 7]],
                            ins=[rs1_in[j].opt()],
                            outs=[rs1_out[j].opt()])

            # ============ post-attention: own 256 tokens ==================
            from contextlib import ExitStack as _ES
            with (
                tc.tile_pool(name="keep2", bufs=1) as keep2,
                tc.tile_pool(name="moe_w", bufs=1) as moe_w,
                tc.tile_pool(name="moe_h", bufs=1) as moe_h,
                tc.tile_pool(name="moe_rt", bufs=1) as moe_rt,
            ):
                # MoE weights (bf16), consolidated DMAs — overlaps G/H/J
                wg_big = moe_w.tile([P, HC, M], BF16, tag="wgB", name="wgB")
                nc.sync.dma_start(
                    out=wg_big, in_=wgT.rearrange("(c p) m -> p c m", p=P))
                wu_big = moe_w.tile([P, HC, M], BF16, tag="wuB", name="wuB")
                nc.sync.dma_start(
                    out=wu_big, in_=wuT.rearrange("(c p) m -> p c m", p=P))
                wg_t = [wg_big[:, hc, :] for hc in range(HC)]
                wu_t = [wu_big[:, hc, :] for hc in range(HC)]
                wdn_t = []
                gw = moe_w.tile([P, HC, E], F32, tag="gw", name="gw")
                nc.sync.dma_start(
                    out=gw.bitcast(F32R),
                    in_=gate_wT.rearrange("(c p) e -> p c e",
                                          p=P).bitcast(F32R))
                gb = moe_w.tile([P, E], F32, tag="gb", name="gb")
                nc.sync.dma_start(out=gb, in_=gate_b.to_broadcast((P, E)))
                ohb = moe_w.tile([P, E], F32, tag="ohb", name="ohb")
                nc.sync.dma_start(out=ohb, in_=oh_row.to_broadcast((P, E)))
                # shared-expert weights too (J must not starve on DMA)
                wsg_big = moe_w.tile([P, HC, M], BF16, tag="wsgB",
                                     name="wsgB")
                nc.sync.dma_start(
                    out=wsg_big, in_=wsgT.rearrange("(c p) m -> p c m", p=P))
                wsu_big = moe_w.tile([P, HC, M], BF16, tag="wsuB",
                                     name="wsuB")
                nc.sync.dma_start(
                    out=wsu_big, in_=wsuT.rearrange("(c p) m -> p c m", p=P))
                wsd_big = moe_w.tile([P, MC, H], BF16, tag="wsdB",
                                     name="wsdB")
                nc.sync.dma_start(
                    out=wsd_big, in_=wsdT.rearrange("(c p) h -> p c h", p=P))
                ws_g = [wsg_big[:, hc, :] for hc in range(HC)]
                ws_u = [wsu_big[:, hc, :] for hc in range(HC)]
                ws_d = [wsd_big[:, mc, :] for mc in range(MC)]

                h_t = [moe_h.tile([P, T], BF16, tag=f"mh{i}",
                                  name=f"mh{i}") for i in range(MC)]
                resid2 = [keep2.tile([P, H], F32, tag=f"res{i}",
                                     name=f"res{i}") for i in range(TC)]
                rt_col = moe_rt.tile([P, T // P], F32, tag="rtc", name="rtc")

                _keepJ_ctx = _ES()
                keepJ = _keepJ_ctx.enter_context(
                    tc.tile_pool(name="keepJ", bufs=1))
                _ps2_ctx = _ES()
                ps_tr2 = _ps2_ctx.enter_context(
                    tc.tile_pool(name="ps_tr2", bufs=2, space="PSUM"))
                ps_mm2 = _ps2_ctx.enter_context(
                    tc.tile_pool(name="ps_mm2", bufs=4, space="PSUM"))

                hidden2 = [keepJ.tile([P, H], F32, tag=f"h2_{i}",
                                      name=f"h2_{i}") for i in range(TC)]
                x2T_own = [keepJ.tile([P, TOK], F32, tag=f"x2T{i}",
                                      name=f"x2T{i}") for i in range(HC)]
                x2b_big = keepJ.tile([P, HC, TOK], BF16, tag="x2bB",
                                     name="x2bB")
                x2T_bf = [x2b_big[:, hc, :] for hc in range(HC)]

                # ---- Phase G: residual + ln2 + transpose (per 128 chunk) --
                with tc.tile_pool(name="g_tmp", bufs=1) as g_tmp:
                    for tcc in range(TC):
                        rsb = g_tmp.tile([P, H], BF16, tag="rsb")
                        nc.sync.dma_start(out=rsb, in_=rs1_out[tcc][:, :])
                        t1 = g_tmp.tile([P, H], F32, tag="gt1")
                        nc.scalar.activation(t1, rsb, ACT.Copy)
                        t2b = g_tmp.tile([P, H], F32, tag="gt2")
                        nc.sync.dma_start(
                            out=t2b, in_=hid_own[tcc * P:(tcc + 1) * P, :])
                        nc.vector.tensor_add(hidden2[tcc], t1, t2b)
                        sq = t2b
                        nc.vector.tensor_mul(sq, hidden2[tcc], hidden2[tcc])
                        ssum = g_tmp.tile([P, 1], F32, tag="ssum")
                        nc.vector.reduce_sum(ssum, sq,
                                             axis=mybir.AxisListType.X)
                        rms2 = g_tmp.tile([P, 1], F32, tag="rms2")
                        nc.scalar.activation(rms2, ssum, ACT.Sqrt,
                                             bias=epsc, scale=invH_col)
                        r2 = g_tmp.tile([P, 1], F32, tag="r2")
                        nc.vector.reciprocal(r2, rms2)
                        x2 = t1
                        nc.scalar.activation(x2, hidden2[tcc], ACT.Copy,
                                             scale=r2)
                        for hc in range(HC):
                            pt = ps_tr2.tile([P, P], F32, tag="tr")
                            nc.tensor.transpose(
                                pt, x2[:, hc * P:(hc + 1) * P], ident)
                            nc.scalar.activation(
                                x2T_own[hc][:, tcc * P:(tcc + 1) * P]
                                .bitcast(F32R), pt, ACT.Copy)
                            nc.scalar.activation(
                                x2T_bf[hc][:, tcc * P:(tcc + 1) * P],
                                pt, ACT.Copy)
                    nc.sync.dma_start(out=ag_in[:, 0:HC * TOK], in_=x2b_big)

                # ---- Phase H: routing on own tokens (f32r decisions) ----
                with tc.tile_pool(name="r_tmp", bufs=2) as r_tmp:
                    for tcc in range(TC):
                        pl = ps_tr2.tile([P, E], F32, tag="tr")
                        for hc in range(HC):
                            _mm(nc, pl, x2T_own[hc][:, tcc * P:(tcc + 1) * P],
                                gw[:, hc, :], start=(hc == 0),
                                stop=(hc == HC - 1))
                        ssig = r_tmp.tile([P, E], F32, tag="ssig")
                        nc.scalar.activation(ssig, pl, ACT.Sigmoid)
                        sb = r_tmp.tile([P, E], F32, tag="sbt")
                        nc.vector.tensor_add(sb, ssig, gb)
                        sbg = sb.rearrange("p (g two) -> p g two", two=2)
                        g4 = r_tmp.tile([P, NG], F32, tag="g4")
                        nc.vector.tensor_add(g4, sbg[:, :, 0], sbg[:, :, 1])
                        m1 = r_tmp.tile([P, 1], F32, tag="m1")
                        nc.vector.reduce_max(m1, g4, axis=mybir.AxisListType.X)
                        eq1 = r_tmp.tile([P, NG], F32, tag="eq1")
                        nc.vector.tensor_scalar(eq1, g4, m1, -BIG,
                                                mybir.AluOpType.is_equal,
                                                mybir.AluOpType.mult)
                        g4b = r_tmp.tile([P, NG], F32, tag="g4b")
                        nc.vector.tensor_add(g4b, g4, eq1)
                        m2 = r_tmp.tile([P, 1], F32, tag="m2")
                        nc.vector.reduce_max(m2, g4b,
                                             axis=mybir.AxisListType.X)
                        gmask = r_tmp.tile([P, NG], F32, tag="gmask")
                        nc.vector.tensor_scalar(gmask, g4, m2, None,
                                                mybir.AluOpType.is_ge)
                        pen = r_tmp.tile([P, E], F32, tag="pen")
                        peng = pen.rearrange("p (g two) -> p g two", two=2)
                        nc.vector.tensor_scalar(peng[:, :, 0], gmask, BIG,
                                                -BIG, mybir.AluOpType.mult,
                                                mybir.AluOpType.add)
                        nc.vector.tensor_scalar(peng[:, :, 1], gmask, BIG,
                                                -BIG, mybir.AluOpType.mult,
                                                mybir.AluOpType.add)
                        ms = r_tmp.tile([P, E], F32, tag="ms")
                        nc.vector.tensor_add(ms, sb, pen)
                        t1 = r_tmp.tile([P, 1], F32, tag="t1")
                        nc.vector.reduce_max(t1, ms, axis=mybir.AxisListType.X)
                        eq2 = r_tmp.tile([P, E], F32, tag="eq2")
                        nc.vector.tensor_scalar(eq2, ms, t1, -BIG,
                                                mybir.AluOpType.is_equal,
                                                mybir.AluOpType.mult)
                        ms2 = r_tmp.tile([P, E], F32, tag="ms2")
                        nc.vector.tensor_add(ms2, ms, eq2)
                        t2r = r_tmp.tile([P, 1], F32, tag="t2r")
                        nc.vector.reduce_max(t2r, ms2,
                                             axis=mybir.AxisListType.X)
                        sel = r_tmp.tile([P, E], F32, tag="sel")
                        nc.vector.tensor_scalar(sel, ms, t2r, None,
                                                mybir.AluOpType.is_ge)
                        wsel = r_tmp.tile([P, E], F32, tag="wsel")
                        nc.vector.tensor_mul(wsel, ssig, sel)
                        dsum = r_tmp.tile([P, 1], F32, tag="dsum")
                        nc.vector.reduce_sum(dsum, wsel,
                                             axis=mybir.AxisListType.X)
                        nc.vector.tensor_scalar_add(dsum, dsum, 1e-20)
                        rdsum = r_tmp.tile([P, 1], F32, tag="rdsum")
                        nc.vector.reciprocal(rdsum, dsum)
                        route = r_tmp.tile([P, E], BF16, tag="route")
                        nc.vector.tensor_scalar(route, wsel, rdsum, None,
                                                mybir.AluOpType.mult)
                        nc.sync.dma_start(
                            out=ag_in[:, HC * TOK + tcc * E:
                                      HC * TOK + (tcc + 1) * E],
                            in_=route)

                # ===== AllGather x2T + routeT across all 8 cores (bf16) ====
                nc.gpsimd.collective_compute(
                    "AllGather", mybir.AluOpType.bypass,
                    replica_groups=[list(range(NCORES))],
                    ins=[ag_in.opt()], outs=[ag_out.opt()])

                # ---- Phase J: shared expert on own tokens -> resid2 ----
                # (bf16, runs while the AllGather is in flight)
                with (
                    tc.tile_pool(name="sh_hs", bufs=1) as sh_hs,
                    tc.tile_pool(name="sh_tmp", bufs=2) as sh_tmp,
                ):
                    hs_t = [sh_hs.tile([P, TOK], BF16, tag=f"hs{i}",
                                       name=f"hs{i}") for i in range(MC)]
                    for mc in range(MC):
                        pg = ps_mm2.tile([P, TOK], F32, tag="mm")
                        for hc in range(HC):
                            nc.tensor.matmul(
                                pg, ws_g[hc][:, mc * P:(mc + 1) * P],
                                x2T_bf[hc], start=(hc == 0),
                                stop=(hc == HC - 1))
                        sg = sh_tmp.tile([P, TOK], F32, tag="sg")
                        nc.scalar.activation(sg, pg, ACT.Silu)
                        pu = ps_mm2.tile([P, TOK], F32, tag="mm")
                        for hc in range(HC):
                            nc.tensor.matmul(
                                pu, ws_u[hc][:, mc * P:(mc + 1) * P],
                                x2T_bf[hc], start=(hc == 0),
                                stop=(hc == HC - 1))
                        nc.vector.tensor_mul(hs_t[mc], pu, sg)
                    for nt in range(NT):
                        pds = [ps_mm2.tile([P, 512], F32, tag="mm",
                                           name=f"pds{_i}")
                               for _i in range(TC)]
                        for mc in range(MC):
                            for tcc in range(TC):
                                nc.tensor.matmul(
                                    pds[tcc],
                                    hs_t[mc][:, tcc * P:(tcc + 1) * P],
                                    ws_d[mc][:, nt * 512:(nt + 1) * 512],
                                    start=(mc == 0), stop=(mc == MC - 1))
                        for tcc in range(TC):
                            nc.vector.tensor_add(
                                resid2[tcc][:, nt * 512:(nt + 1) * 512],
                                pds[tcc],
                                hidden2[tcc][:, nt * 512:(nt + 1) * 512])

                # free J-phase SBUF + the 6 PSUM banks before K1
                _keepJ_ctx.close()
                _ps2_ctx.close()

                # ---- Phase K1: gate/up for all 2048 tokens (bf16) ----
                with (
                    tc.tile_pool(name="moe_xb", bufs=1) as moe_xb,
                    tc.tile_pool(name="k_sg", bufs=2) as k_sg,
                    tc.tile_pool(name="ps_g", bufs=4, space="PSUM") as ps_g,
                    tc.tile_pool(name="ps_u", bufs=4, space="PSUM") as ps_u,
                ):
                    # xq/routes/wdn loads ride the scalar-engine HWDGE queue
                    # so they can't block other traffic behind the AllGather.
                    # ag_out block tb is [128, AGCOLS] partition-major: one
                    # DMA per block for x, one tiny one for the routes.
                    xq_big = moe_xb.tile([P, HC, T], BF16, tag="xqB",
                                         name="xqB")
                    rnat = moe_xb.tile([P, TBLK, TC, E], BF16, tag="rnat",
                                       name="rnat")
                    for tb in range(TBLK):
                        blk = ag_out[tb * P:(tb + 1) * P, :]
                        nc.scalar.dma_start(
                            out=xq_big[:, :, tb * TOK:(tb + 1) * TOK],
                            in_=blk[:, 0:HC * TOK].rearrange(
                                "p (c t) -> p c t", t=TOK))
                        nc.scalar.dma_start(
                            out=rnat[:, tb, :, :],
                            in_=blk[:, HC * TOK:AGCOLS].rearrange(
                                "p (c e) -> p c e", e=E))
                    xq = [xq_big[:, hc, :] for hc in range(HC)]
                    for mc in range(MC):
                        # lives in moe_h pool: must survive into K2
                        twd = moe_h.tile([P, H], BF16, tag=f"wdn{mc}",
                                         name=f"wdn{mc}")
                        nc.scalar.dma_start(out=twd,
                                            in_=wdnT[mc * P:(mc + 1) * P, :])
                        wdn_t.append(twd)
                    # route column for this core's expert per 128-token chunk
                    with tc.tile_pool(name="kp_tmp", bufs=2) as kp_tmp:
                        for tb in range(TBLK):
                            rf = kp_tmp.tile([P, TC, E], F32, tag="rf")
                            nc.scalar.activation(rf, rnat[:, tb, :, :],
                                                 ACT.Copy)
                            for tcc in range(TC):
                                rsel = kp_tmp.tile([P, E], F32, tag="rsel")
                                nc.vector.tensor_mul(rsel, rf[:, tcc, :], ohb)
                                idx = tb * TC + tcc
                                nc.vector.reduce_sum(
                                    rt_col[:, idx:idx + 1], rsel,
                                    axis=mybir.AxisListType.X)

                    for mc in range(MC):
                        pgs = [ps_g.tile([P, 512], F32, tag="g",
                                         name=f"pg{mc}_{q}")
                               for q in range(TQ)]
                        pus = [ps_u.tile([P, 512], F32, tag="u",
                                         name=f"pu{mc}_{q}")
                               for q in range(TQ)]
                        for hc in range(HC):
                            wslc = wg_t[hc][:, mc * P:(mc + 1) * P]
                            for q in range(TQ):
                                nc.tensor.matmul(
                                    pgs[q], wslc,
                                    xq[hc][:, q * 512:(q + 1) * 512],
                                    start=(hc == 0), stop=(hc == HC - 1))
                        for hc in range(HC):
                            wslc = wu_t[hc][:, mc * P:(mc + 1) * P]
                            for q in range(TQ):
                                nc.tensor.matmul(
                                    pus[q], wslc,
                                    xq[hc][:, q * 512:(q + 1) * 512],
                                    start=(hc == 0), stop=(hc == HC - 1))
                        for q in range(TQ):
                            sg = k_sg.tile([P, 512], F32, tag="ksg")
                            nc.scalar.activation(sg, pgs[q], ACT.Silu)
                            nc.vector.tensor_mul(
                                h_t[mc][:, q * 512:(q + 1) * 512],
                                pus[q], sg)

                # ---- Phase K2: down proj by h' column chunks; chunked
                #      column ReduceScatters pipeline behind the compute ----
                with (
                    tc.tile_pool(name="ps_d", bufs=6, space="PSUM") as ps_d,
                    tc.tile_pool(name="moe_st", bufs=6) as moe_st,
                ):
                    for nt in range(NT):
                        for tc16 in range(T // P):
                            pd = ps_d.tile([P, 512], F32, tag="d")
                            for mc in range(MC):
                                nc.tensor.matmul(
                                    pd,
                                    h_t[mc][:, tc16 * P:(tc16 + 1) * P],
                                    wdn_t[mc][:, nt * 512:(nt + 1) * 512],
                                    start=(mc == 0), stop=(mc == MC - 1))
                            st = moe_st.tile([P, 512], BF16, tag="mst")
                            # route weight folded in as per-token row scale
                            nc.scalar.activation(
                                st, pd, ACT.Copy,
                                scale=rt_col[:, tc16:tc16 + 1])
                            nc.sync.dma_start(
                                out=rs2_in[nt][tc16 * P:(tc16 + 1) * P, :],
                                in_=st)
                        nc.gpsimd.collective_compute(
                            "ReduceScatter", mybir.AluOpType.add,
                            replica_groups=[list(range(NCORES))],
                            ins=[rs2_in[nt].opt()],
                            outs=[rs2_out[nt].opt()])

                # ===== final: own rows = RS output + resid2 =====
                with tc.tile_pool(name="fin", bufs=2) as fin:
                    out_r = out.rearrange("(c p) h -> p c h", p=P)
                    for nt in range(NT):
                        mo = fin.tile([P, TC, 512], BF16, tag="mo")
                        nc.sync.dma_start(
                            out=mo,
                            in_=rs2_out[nt].rearrange("(c p) m -> p c m",
                                                      p=P))
                        fo = fin.tile([P, TC, 512], F32, tag="fo")
                        for tcc in range(TC):
                            mof = fin.tile([P, 512], F32, tag="mof")
                            nc.scalar.activation(mof, mo[:, tcc, :], ACT.Copy)
                            nc.vector.tensor_add(
                                fo[:, tcc, :], mof,
                                resid2[tcc][:, nt * 512:(nt + 1) * 512])
                        nc.sync.dma_start(
                            out=out_r[:, :, nt * 512:(nt + 1) * 512],
                            in_=fo)

    nc.compile()
    return nc


def prep_inputs(c, inputs):
    """Build the per-core input map (all host-side layout/weight-fold work)."""
    f = np.float32
    bf = ml_dtypes.bfloat16
    hs = np.asarray(inputs["hidden_states"], f)
    cos = np.asarray(inputs["cos"], f)
    sin = np.asarray(inputs["sin"], f)
    ln1 = np.asarray(inputs["ln1_w"], f)
    w_qkv = np.asarray(inputs["w_qkv"], f)
    q_ln = np.asarray(inputs["q_ln_w"], f)
    k_ln = np.asarray(inputs["k_ln_w"], f)
    w_dense = np.asarray(inputs["w_dense"], f)
    ln2 = np.asarray(inputs["ln2_w"], f)
    gate_w = np.asarray(inputs["gate_w"], f)
    gate_b = np.asarray(inputs["gate_bias"], f)
    we_gate = np.asarray(inputs["we_gate"], f)
    we_up = np.asarray(inputs["we_up"], f)
    we_down = np.asarray(inputs["we_down"], f)
    ws_gate = np.asarray(inputs["ws_gate"], f)
    ws_up = np.asarray(inputs["ws_up"], f)
    ws_down = np.asarray(inputs["ws_down"], f)

    b, kv, e = c // 4, c % 4, c
    perm = np.concatenate([np.arange(64, 128), np.arange(64)])
    sign = np.concatenate([-np.ones(64, f), np.ones(64, f)])

    q_rows = np.arange(kv * QH * HD, (kv + 1) * QH * HD)
    k_rows = np.arange(NH * HD + kv * HD, NH * HD + (kv + 1) * HD)
    v_rows = np.arange(NH * HD + NKV * HD + kv * HD,
                       NH * HD + NKV * HD + (kv + 1) * HD)
    sel = np.concatenate([q_rows, k_rows, v_rows])
    w_sel = w_qkv[sel] * ln1[None, :]

    oh_row = np.zeros((1, E), f)
    oh_row[0, e] = 1.0

    C = np.ascontiguousarray
    # core owns tokens [128r, 128r+128) and [512+128r, 512+128r+128) of its
    # batch (r = kv group rank) — the RS1 chunk scatter order
    own_rows = np.concatenate([np.arange(kv * P, (kv + 1) * P),
                               np.arange(512 + kv * P, 512 + (kv + 1) * P)])
    return {
        "hT": C(hs[b].T),
        "hid_own": C(hs[b, own_rows]),
        "cos_qw": C(cos[b].T * q_ln[:, None]),
        "sin_qw": C(sin[b].T * (q_ln[perm] * sign)[:, None]),
        "cos_kw": C(cos[b].T * k_ln[:, None]),
        "sin_kw": C(sin[b].T * (k_ln[perm] * sign)[:, None]),
        "wqkvT": C(w_sel.T),
        "wdT": C(w_dense[:, kv * QH * HD:(kv + 1) * QH * HD].T),
        "gate_wT": C((gate_w * ln2[None, :]).T),
        "gate_b": C(gate_b.reshape(1, E)),
        "oh_row": oh_row,
        "wgT": C((we_gate[e] * ln2[None, :]).T).astype(bf),
        "wuT": C((we_up[e] * ln2[None, :]).T).astype(bf),
        "wdnT": C(we_down[e].T).astype(bf),
        "wsgT": C((ws_gate * ln2[None, :]).T).astype(bf),
        "wsuT": C((ws_up * ln2[None, :]).T).astype(bf),
        "wsdT": C(ws_down.T).astype(bf),
    }


_NC_CACHE = {}


def get_program():
    if "nc" not in _NC_CACHE:
        _NC_CACHE["nc"] = build_program()
    return _NC_CACHE["nc"]


def kernel(**inputs) -> np.ndarray:
    from concourse.bass_utils import run_bass_kernel_spmd

    nc = get_program()
    in_maps = [prep_inputs(c, inputs) for c in range(NCORES)]
    res = run_bass_kernel_spmd(nc, in_maps, core_ids=list(range(NCORES)))
    full = np.empty((T, H), np.float32)
    for c in range(NCORES):
        b, r = c // 4, c % 4
        sl = res.results[c]["out"]
        full[b * S + r * P:b * S + (r + 1) * P] = sl[:P]
        full[b * S + 512 + r * P:b * S + 512 + (r + 1) * P] = sl[P:]
    return full.reshape(B, S, H).astype(np.float32)
